# revision 4
# baseline (speedup 1.0000x reference)
"""Trainium2 Bass kernel for the ContinuousRNN problem (z-space rewrite).

Reference computation (per batch row b):
    h_0 = 0                         # [N], N=100
    z_t = W_rec @ h_t + W_in @ u_t  # u_t = inputs[b, t] (3-dim)
    h_{t+1} = (1-DT)*h_t + DT*tanh(z_t) + NOISE_STD*nu_t
    out_t = W_out @ h_{t+1}         # 3-dim

Instead of tracking h, track s_t = [z_t; o_t] (o_t = W_out @ h_t, 103 rows)
in PSUM.  Substituting h out of the recurrence:

    s_{t+1} = 0.85*s_t + S@[th_t;0] + d_t,      th_t = tanh(z_t)
    S   = DT*[W_rec; W_out]  (+ W_in rows, see make_s_mat)
    d_t = NOISE_STD*[W_rec; W_out]@nu_t + [x_{t+1}-0.85*x_t; 0]

d_t is input-only, so it is precomputed on the HOST (one big sgemm) and
DMA-streamed to an SBUF ring; outputs o_t stream back per step.

Per-step critical path is only 2 engine hops:
    ACT: th_t = tanh(s_t[0:100])  PSUM -> SBUF fp16
    PE:  s_{t+1} = a_t + S@[th_t;0]   (matmul start=False accumulating
         onto a_t, which DVE pre-writes into the target PSUM bank;
         the bank's has_written bits were set once by a warmup matmul
         and are never cleared - walrus's own "dummy matmul" trick)
Off-chain: DVE computes a_t = 0.85*s_t + d_t (PSUM+SBUF -> PSUM) and
copies o_t to the output staging buffer.  The stationary weight matrix
is loaded exactly once (all matmuls carry ldweights=False).

Data-parallel over batch across 8 NeuronCores (64 rows/core).
"""

import sys

for _p in ("/opt/trn_rl_repo",):
    if _p not in sys.path:
        sys.path.insert(0, _p)

import numpy as np

import concourse.bass as bass
import concourse.bacc as bacc
import concourse.mybir as mybir
from concourse import tile
from concourse.bass_utils import run_bass_kernel_spmd

F32 = mybir.dt.float32
F16 = mybir.dt.float16

N = 100          # hidden size
NB = 3           # n_bits
K = N + NB       # state/contraction size (103)
B = 512          # full batch
T = 2048         # time steps
NCORES = 8
BL = B // NCORES  # batch per core (64)
DT = np.float32(0.15)
NOISE_STD = np.float32(0.015)
DECAY = np.float32(1.0) - DT  # 0.85


def _round_up_pe(size):
    for v in (32, 64, 128):
        if v >= size:
            return v
    raise AssertionError(size)


def matmul_noldw(nc, out, lhsT, rhs, start, stop, skip_group_check=False):
    """nc.tensor.matmul without the implicit per-matmul LDWEIGHTS.

    Mirrors BassTensorEngine.matmul for the plain case (no perf mode /
    transpose / quant), but sets InstMatmult.ldweights=False so codegen
    reuses the stationary loaded by an earlier nc.tensor.ldweights().
    """
    q = nc.tensor
    assert out.space == bass.MemorySpace.PSUM
    assert lhsT.space == bass.MemorySpace.SBUF
    assert rhs.space == bass.MemorySpace.SBUF
    assert lhsT.partition_size() == rhs.partition_size()
    assert out.partition_size() == lhsT.free_size()
    assert out.free_size() == rhs.free_size()

    keep_dims = {0}
    ifmap_ap = q.lower_ap(rhs.opt(keep_dims), opt=False)
    weights_ap = q.lower_ap(lhsT.opt(keep_dims), opt=False,
                            for_matmul_weights=True)
    out_ap = q.lower_ap(out)
    tile_size = (_round_up_pe(rhs.partition_size()),
                 _round_up_pe(out.partition_size()))
    assert lhsT.base_partition() == rhs.base_partition()
    tile_position = (lhsT.base_partition(), out.base_partition())
    return q.add_instruction(
        mybir.InstMatmult(
            name=q.bass.get_next_instruction_name(),
            replication_resolution=0,
            replication_shift_amnt=0,
            replication_num_rows=0,
            start_tensor_calc=start,
            stop_tensor_calc=stop,
            ins=[ifmap_ap, weights_ap],
            outs=[out_ap],
            perf_mode=None,
            is_transpose=None,
            ifmap_quant_offset=None,
            weights_quant_offset=None,
            bass_skip_group_check=skip_group_check,
            tile_position=tile_position,
            tile_size=tile_size,
        )
    )


def emit_rnn(tc, nc, aps, *, t_steps=T, dchunk=128, sbanks=4, th_slots=4,
             odma=64, ldw_per_mm=False, ablate=""):
    """Emit the unrolled z-space RNN scan.

    aps: DRAM APs: s_mat [K,K] f16 (lhsT stationary), r0 [K,BL] f16,
         d [K, t_steps*BL] f32 (host-precomputed drive, time-major),
         out_t [NB, (t_steps+1)*BL] f32.
    """
    assert t_steps % odma == 0
    assert t_steps % dchunk == 0 or t_steps < dchunk
    n_chunks = (t_steps + dchunk - 1) // dchunk
    oslots = 2 * odma
    mult = mybir.AluOpType.mult
    add = mybir.AluOpType.add
    tanh = mybir.ActivationFunctionType.Tanh

    cpool = tc.alloc_tile_pool(name="const", bufs=1)
    tpool = tc.alloc_tile_pool(name="th", bufs=1)
    dpool = tc.alloc_tile_pool(name="dring", bufs=1)
    opool = tc.alloc_tile_pool(name="ostg", bufs=1)
    ppool = tc.alloc_tile_pool(name="psum", bufs=1, space="PSUM")

    s_sb = cpool.tile([K, K], F16, name="s_sb")
    r0_sb = cpool.tile([K, BL], F16, name="r0_sb")
    nc.sync.dma_start(s_sb[:, :], aps["s_mat"][:, :])
    nc.sync.dma_start(r0_sb[:, :], aps["r0"][:, :])

    # tanh outputs; rows 100:103 stay zero forever (memset below)
    th = tpool.tile([K, th_slots * BL], F16, name="th")
    nc.vector.memset(th[:, :], 0.0)

    # host-precomputed d_t ring (DMA, 3 chunks)
    dr = dpool.tile([K, 3 * dchunk * BL], F32, name="dr")

    # output staging: rows 4:7 hold o (copied from psum rows 96:103)
    ostg = opool.tile([7, oslots * BL], F32, name="ostg")

    # PSUM state ring
    sb = [ppool.tile([128, 512], F32, name=f"sb{i}") for i in range(sbanks)]

    def mm(out, rhs, start, stop, skip=False):
        if ldw_per_mm:
            nc.tensor.matmul(out, s_sb[:, :], rhs, start=start, stop=stop,
                             skip_group_check=skip)
        else:
            matmul_noldw(nc, out, s_sb[:, :], rhs, start, stop,
                         skip_group_check=skip)

    def d_dma(c):
        if c >= n_chunks:
            return
        slot = c % 3
        c0, c1 = c * dchunk * BL, min((c + 1) * dchunk, t_steps) * BL
        nc.sync.dma_start(dr[:, slot * dchunk * BL:
                             slot * dchunk * BL + (c1 - c0)],
                          aps["d"][:, c0:c1])

    if not ldw_per_mm:
        nc.tensor.ldweights(s_sb[:, :])

    d_dma(0)
    d_dma(1)

    # s_0 = S.T @ [0;u_0] = [W_in@u_0; 0] into bank 0 (sets has_written);
    # dummy warmups set the bits in the other s banks.
    mm(sb[0][0:K, 0:BL], r0_sb[:, :], start=True, stop=True)
    for i in range(1, sbanks):
        mm(sb[i][0:K, 0:BL], th[:, 0:BL], start=True, stop=True)

    for t in range(t_steps + 1):
        bank = sb[t % sbanks]
        if t < t_steps:
            ths = (t % th_slots) * BL
            if ablate != "notanh":
                nc.scalar.activation(th[0:N, ths:ths + BL],
                                     bank[0:N, 0:BL], tanh)
            nxt = sb[(t + 1) % sbanks]
            dsl = ((t // dchunk) % 3) * dchunk + (t % dchunk)
            # a_t = 0.85*s_t + d_t -> next psum bank (off critical path)
            nc.vector.scalar_tensor_tensor(
                nxt[0:K, 0:BL], bank[0:K, 0:BL], float(DECAY),
                dr[:, dsl * BL:(dsl + 1) * BL], mult, add)
        if t >= 1:
            osl = ((t - 1) % oslots) * BL
            nc.vector.tensor_copy(ostg[:, osl:osl + BL], bank[96:K, 0:BL])
            if t % odma == 0:
                s0 = (t - odma) % oslots
                nc.sync.dma_start(
                    aps["out_t"].rearrange("p (t b) -> p t b", b=BL)[
                        :, t - odma + 1:t + 1, :],
                    ostg.rearrange("p (s b) -> p s b", b=BL)[
                        4:7, s0:s0 + odma, :])
        if t < t_steps:
            ths = (t % th_slots) * BL
            # notanh ablation: rhs = constant zero slot, no ACT dep
            src = th[:, 0:BL] if ablate == "notanh" else th[:, ths:ths + BL]
            mm(nxt[0:K, 0:BL], src, start=False, stop=True, skip=True)
            if t % dchunk == 0:
                d_dma(t // dchunk + 2)

    for p in (ppool, opool, dpool, tpool, cpool):
        p.release()


def build_nc(*, t_steps=T, dchunk=128, sbanks=4, th_slots=4, odma=64,
             num_devices=NCORES, ldw_per_mm=False, ablate=""):
    nc = bacc.Bacc("TRN2", target_bir_lowering=False, debug=False,
                   num_devices=num_devices)
    aps = {
        "s_mat": nc.dram_tensor("s_mat", [K, K], F16,
                                kind="ExternalInput").ap(),
        "r0": nc.dram_tensor("r0", [K, BL], F16, kind="ExternalInput").ap(),
        "d": nc.dram_tensor("d", [K, t_steps * BL], F32,
                            kind="ExternalInput").ap(),
        "out_t": nc.dram_tensor("out_t", [NB, (t_steps + 1) * BL], F32,
                                kind="ExternalOutput").ap(),
    }
    with tile.TileContext(nc) as tcx:
        emit_rnn(tcx, nc, aps, t_steps=t_steps, dchunk=dchunk, sbanks=sbanks,
                 th_slots=th_slots, odma=odma, ldw_per_mm=ldw_per_mm,
                 ablate=ablate)
    nc.compile()
    return nc


def make_s_mat(recurrent_weights, input_weights, output_weights):
    """lhsT stationary [K, K]: rows 0:100 contract tanh(h) through
    DT*[W_rec; W_out]; rows 100:103 contract the initial input drive
    through W_in (z rows only, used by the s_0 init matmul)."""
    st = np.zeros((K, K), np.float32)
    st[:N, :N] = DT * recurrent_weights.T
    st[:N, N:] = DT * output_weights.T
    st[N:, :N] = input_weights.T
    return st.astype(np.float16)


def make_in_maps(inputs, noise, recurrent_weights, input_weights,
                 output_weights, *, t_steps=T, bl=BL, ncores=NCORES):
    s = make_s_mat(recurrent_weights, input_weights, output_weights)
    b = ncores * bl
    w103 = np.concatenate([recurrent_weights, output_weights], 0)  # [K, N]
    # d = NOISE_STD*[W_rec;W_out]@nu + [x_{t+1}-0.85*x_t; 0], all (b,t):
    nu = noise.reshape(b * t_steps, N).astype(np.float32)
    d_noise = (nu @ (NOISE_STD * w103).T)           # [b*t, K]
    x = (inputs.reshape(b * t_steps, NB).astype(np.float32)
         @ input_weights.T).reshape(b, t_steps, N)  # [b, t, N]
    xd = -DECAY * x
    xd[:, :-1] += x[:, 1:]
    d_all = d_noise.reshape(b, t_steps, K)
    d_all[:, :, :N] += xd
    in_maps = []
    for c in range(ncores):
        bs = slice(c * bl, (c + 1) * bl)
        dt_c = np.ascontiguousarray(
            d_all[bs].transpose(2, 1, 0)).reshape(K, t_steps * bl)
        r0 = np.zeros((K, bl), np.float16)
        r0[N:] = inputs[bs, 0].astype(np.float16).T
        in_maps.append({"s_mat": s, "r0": r0, "d": dt_c})
    return in_maps


def gather_out(results, *, t_steps=T, bl=BL, ncores=NCORES):
    out = np.empty((ncores * bl, t_steps, NB), np.float32)
    for c in range(ncores):
        ot = results[c]["out_t"].reshape(NB, t_steps + 1, bl)
        out[c * bl:(c + 1) * bl] = ot[:, 1:, :].transpose(2, 1, 0)
    return out


_NC_CACHE = {}


def kernel(inputs, noise, recurrent_weights, input_weights, output_weights,
           **run_kwargs):
    cfg = run_kwargs.pop("cfg", {})
    key = tuple(sorted(cfg.items()))
    if key not in _NC_CACHE:
        _NC_CACHE[key] = build_nc(**cfg)
    nc = _NC_CACHE[key]
    in_maps = make_in_maps(inputs, noise, recurrent_weights, input_weights,
                           output_weights)
    res = run_bass_kernel_spmd(nc, in_maps, core_ids=list(range(NCORES)),
                               **run_kwargs)
    out = gather_out(res.results)
    if run_kwargs.get("trace"):
        return out, res
    return out


# revision 5
# speedup vs baseline: 1.6500x; 1.6500x over previous
"""Trainium2 Bass kernel for the ContinuousRNN problem (z-space rewrite).

Reference computation (per batch row b):
    h_0 = 0                         # [N], N=100
    z_t = W_rec @ h_t + W_in @ u_t  # u_t = inputs[b, t] (3-dim)
    h_{t+1} = (1-DT)*h_t + DT*tanh(z_t) + NOISE_STD*nu_t
    out_t = W_out @ h_{t+1}         # 3-dim

Instead of tracking h, track s_t = [z_t; o_t] (o_t = W_out @ h_t, 103 rows)
in PSUM.  Substituting h out of the recurrence:

    s_{t+1} = 0.85*s_t + S@[th_t;0] + d_t,      th_t = tanh(z_t)
    S   = DT*[W_rec; W_out]  (+ W_in rows, see make_s_mat)
    d_t = NOISE_STD*[W_rec; W_out]@nu_t + [x_{t+1}-0.85*x_t; 0]

d_t is input-only, so it is precomputed on the HOST (one big sgemm) and
DMA-streamed (fp16, split across DMA queues); outputs stream back.

Per-step critical path is 2 engine hops:
    ACT: th_t = tanh(A_t[0:100])  PSUM -> SBUF fp16
    PE:  A_{t+1} = a_t + S@[th_t;0]   (matmul start=False accumulating
         onto a_t, which DVE pre-writes into the target PSUM bank; the
         bank's has_written bits were set once by a warmup matmul and
         never cleared - walrus's own "dummy matmul" trick)

The state lives in TWO psum bank families: A-ring is read ONLY by the
ACT (tanh) and B-ring ONLY by the DVE (a_t STT, output copy), because
ScalarE and VectorE accesses to the SAME psum bank serialize (Tile
emits cross-engine sems for same-bank access, and the hardware faults
on true concurrency).  The PE writes both copies (two matmuls per
step); DVE pre-writes a_t into both A_{t+1} and B_{t+1}.

Data-parallel over batch across 8 NeuronCores (64 rows/core).
"""

import sys

for _p in ("/opt/trn_rl_repo",):
    if _p not in sys.path:
        sys.path.insert(0, _p)

import numpy as np

import concourse.bass as bass
import concourse.bacc as bacc
import concourse.mybir as mybir
from concourse import tile
from concourse.bass_utils import run_bass_kernel_spmd

F32 = mybir.dt.float32
F16 = mybir.dt.float16

N = 100          # hidden size
NB = 3           # n_bits
K = N + NB       # state/contraction size (103)
B = 512          # full batch
T = 2048         # time steps
NCORES = 8
BL = B // NCORES  # batch per core (64)
DT = np.float32(0.15)
NOISE_STD = np.float32(0.015)
DECAY = np.float32(1.0) - DT  # 0.85


def _round_up_pe(size):
    for v in (32, 64, 128):
        if v >= size:
            return v
    raise AssertionError(size)


def matmul_noldw(nc, out, lhsT, rhs, start, stop, skip_group_check=False):
    """nc.tensor.matmul with InstMatmult.ldweights=False: codegen reuses
    the stationary already in the PE array instead of reloading it.
    (bass may still emit an Ldweights as a carrier for a second sem wait;
    it issues concurrently with the matmul and reloads the same matrix.)
    """
    q = nc.tensor
    assert out.space == bass.MemorySpace.PSUM
    assert lhsT.space == bass.MemorySpace.SBUF
    assert rhs.space == bass.MemorySpace.SBUF
    assert lhsT.partition_size() == rhs.partition_size()
    assert out.partition_size() == lhsT.free_size()
    assert out.free_size() == rhs.free_size()

    keep_dims = {0}
    ifmap_ap = q.lower_ap(rhs.opt(keep_dims), opt=False)
    weights_ap = q.lower_ap(lhsT.opt(keep_dims), opt=False,
                            for_matmul_weights=True)
    out_ap = q.lower_ap(out)
    tile_size = (_round_up_pe(rhs.partition_size()),
                 _round_up_pe(out.partition_size()))
    assert lhsT.base_partition() == rhs.base_partition()
    tile_position = (lhsT.base_partition(), out.base_partition())
    return q.add_instruction(
        mybir.InstMatmult(
            name=q.bass.get_next_instruction_name(),
            replication_resolution=0,
            replication_shift_amnt=0,
            replication_num_rows=0,
            start_tensor_calc=start,
            stop_tensor_calc=stop,
            ins=[ifmap_ap, weights_ap],
            outs=[out_ap],
            perf_mode=None,
            is_transpose=None,
            ifmap_quant_offset=None,
            weights_quant_offset=None,
            bass_skip_group_check=skip_group_check,
            tile_position=tile_position,
            tile_size=tile_size,
        )
    )


def emit_rnn(tc, nc, aps, *, t_steps=T, dchunk=128, dsplit=4, abanks=4,
             bbanks=3, th_slots=4, odma=64, orings=4, ldw_per_mm=False,
             ablate=""):
    """Emit the unrolled z-space RNN scan (dual psum bank families).

    aps: DRAM APs: s_mat [K,K] f16 (lhsT stationary), r0 [K,BL] f16,
         d [K, t_steps*BL] f16 (host-precomputed drive, time-major),
         out_t [NB, (t_steps+1)*BL] f32.
    """
    assert t_steps % odma == 0
    assert t_steps % dchunk == 0 or t_steps < dchunk
    assert (dchunk * BL) % dsplit == 0
    n_chunks = (t_steps + dchunk - 1) // dchunk
    oslots = orings * odma
    mult = mybir.AluOpType.mult
    add = mybir.AluOpType.add
    tanh = mybir.ActivationFunctionType.Tanh

    cpool = tc.alloc_tile_pool(name="const", bufs=1)
    tpool = tc.alloc_tile_pool(name="th", bufs=1)
    dpool = tc.alloc_tile_pool(name="dring", bufs=1)
    opool = tc.alloc_tile_pool(name="ostg", bufs=1)
    ppool = tc.alloc_tile_pool(name="psum", bufs=1, space="PSUM")

    s_sb = cpool.tile([K, K], F16, name="s_sb")
    r0_sb = cpool.tile([K, BL], F16, name="r0_sb")
    nc.sync.dma_start(s_sb[:, :], aps["s_mat"][:, :])
    nc.sync.dma_start(r0_sb[:, :], aps["r0"][:, :])

    # tanh outputs; rows 100:103 stay zero forever (memset below)
    th = tpool.tile([K, th_slots * BL], F16, name="th")
    nc.vector.memset(th[:, :], 0.0)

    # host-precomputed d_t ring (DMA, 3 chunks, fp16)
    dr = dpool.tile([K, 3 * dchunk * BL], F16, name="dr")

    # output staging: rows 4:7 hold o (copied from psum rows 96:103)
    ostg = opool.tile([7, oslots * BL], F32, name="ostg")

    # PSUM state rings: A read by ACT only, B read by DVE only
    sa = [ppool.tile([128, 512], F32, name=f"sa{i}") for i in range(abanks)]
    sb = [ppool.tile([128, 512], F32, name=f"sb{i}") for i in range(bbanks)]

    def mm(out, rhs, start, stop, skip=False):
        if ldw_per_mm:
            nc.tensor.matmul(out, s_sb[:, :], rhs, start=start, stop=stop,
                             skip_group_check=skip)
        else:
            matmul_noldw(nc, out, s_sb[:, :], rhs, start, stop,
                         skip_group_check=skip)

    def d_dma(c):
        if c >= n_chunks:
            return
        slot = c % 3
        cols = min((c + 1) * dchunk, t_steps) * BL - c * dchunk * BL
        piece = cols // dsplit
        for i in range(dsplit):
            nc.sync.dma_start(
                dr[:, slot * dchunk * BL + i * piece:
                   slot * dchunk * BL + (i + 1) * piece],
                aps["d"][:, c * dchunk * BL + i * piece:
                         c * dchunk * BL + (i + 1) * piece])

    if not ldw_per_mm:
        nc.tensor.ldweights(s_sb[:, :])

    d_dma(0)
    d_dma(1)

    # s_0 = S.T @ [0;u_0] = [W_in@u_0; 0] into A0/B0 (sets has_written);
    # dummy warmups set the bits in the other s banks.
    mm(sa[0][0:K, 0:BL], r0_sb[:, :], start=True, stop=True)
    mm(sb[0][0:K, 0:BL], r0_sb[:, :], start=True, stop=True)
    for i in range(1, abanks):
        mm(sa[i][0:K, 0:BL], th[:, 0:BL], start=True, stop=True)
    for i in range(1, bbanks):
        mm(sb[i][0:K, 0:BL], th[:, 0:BL], start=True, stop=True)

    for t in range(t_steps + 1):
        banka = sa[t % abanks]
        bankb = sb[t % bbanks]
        if t < t_steps:
            ths = (t % th_slots) * BL
            if ablate != "notanh":
                nc.scalar.activation(th[0:N, ths:ths + BL],
                                     banka[0:N, 0:BL], tanh)
            nxta = sa[(t + 1) % abanks]
            nxtb = sb[(t + 1) % bbanks]
            dsl = ((t // dchunk) % 3) * dchunk + (t % dchunk)
            # a_t = 0.85*s_t + d_t pre-written into both next banks
            # (off critical path; reads the DVE-family B bank)
            nc.vector.scalar_tensor_tensor(
                nxta[0:K, 0:BL], bankb[0:K, 0:BL], float(DECAY),
                dr[:, dsl * BL:(dsl + 1) * BL], mult, add)
            nc.vector.scalar_tensor_tensor(
                nxtb[0:K, 0:BL], bankb[0:K, 0:BL], float(DECAY),
                dr[:, dsl * BL:(dsl + 1) * BL], mult, add)
        if t >= 1:
            osl = ((t - 1) % oslots) * BL
            nc.vector.tensor_copy(ostg[:, osl:osl + BL], bankb[96:K, 0:BL])
            if t % odma == 0:
                s0 = ((t - odma) % oslots) * BL
                # contiguous on both sides: 1 descriptor per partition
                nc.sync.dma_start(
                    aps["out_t"][:, (t - odma + 1) * BL:(t + 1) * BL],
                    ostg[4:7, s0:s0 + odma * BL])
        if t < t_steps:
            ths = (t % th_slots) * BL
            # notanh ablation: rhs = constant zero slot, no ACT dep
            src = th[:, 0:BL] if ablate == "notanh" else th[:, ths:ths + BL]
            mm(nxta[0:K, 0:BL], src, start=False, stop=True, skip=True)
            mm(nxtb[0:K, 0:BL], src, start=False, stop=True, skip=True)
            if t % dchunk == 0:
                d_dma(t // dchunk + 2)

    for p in (ppool, opool, dpool, tpool, cpool):
        p.release()


def build_nc(*, t_steps=T, dchunk=128, dsplit=4, abanks=4, bbanks=3,
             th_slots=4, odma=64, orings=4, num_devices=NCORES,
             ldw_per_mm=False, ablate=""):
    nc = bacc.Bacc("TRN2", target_bir_lowering=False, debug=False,
                   num_devices=num_devices)
    aps = {
        "s_mat": nc.dram_tensor("s_mat", [K, K], F16,
                                kind="ExternalInput").ap(),
        "r0": nc.dram_tensor("r0", [K, BL], F16, kind="ExternalInput").ap(),
        "d": nc.dram_tensor("d", [K, t_steps * BL], F16,
                            kind="ExternalInput").ap(),
        "out_t": nc.dram_tensor("out_t", [NB, (t_steps + 1) * BL], F32,
                                kind="ExternalOutput").ap(),
    }
    with tile.TileContext(nc) as tcx:
        emit_rnn(tcx, nc, aps, t_steps=t_steps, dchunk=dchunk, dsplit=dsplit,
                 abanks=abanks, bbanks=bbanks, th_slots=th_slots, odma=odma,
                 orings=orings, ldw_per_mm=ldw_per_mm, ablate=ablate)
    nc.compile()
    return nc


def make_s_mat(recurrent_weights, input_weights, output_weights):
    """lhsT stationary [K, K]: rows 0:100 contract tanh(h) through
    DT*[W_rec; W_out]; rows 100:103 contract the initial input drive
    through W_in (z rows only, used by the s_0 init matmul)."""
    st = np.zeros((K, K), np.float32)
    st[:N, :N] = DT * recurrent_weights.T
    st[:N, N:] = DT * output_weights.T
    st[N:, :N] = input_weights.T
    return st.astype(np.float16)


def make_in_maps(inputs, noise, recurrent_weights, input_weights,
                 output_weights, *, t_steps=T, bl=BL, ncores=NCORES):
    s = make_s_mat(recurrent_weights, input_weights, output_weights)
    b = ncores * bl
    w103 = np.concatenate([recurrent_weights, output_weights], 0)  # [K, N]
    # d = NOISE_STD*[W_rec;W_out]@nu + [x_{t+1}-0.85*x_t; 0], all (b,t):
    nu = noise.reshape(b * t_steps, N).astype(np.float32)
    d_noise = (nu @ (NOISE_STD * w103).T)           # [b*t, K]
    x = (inputs.reshape(b * t_steps, NB).astype(np.float32)
         @ input_weights.T).reshape(b, t_steps, N)  # [b, t, N]
    xd = -DECAY * x
    xd[:, :-1] += x[:, 1:]
    d_all = d_noise.reshape(b, t_steps, K)
    d_all[:, :, :N] += xd
    d_all = d_all.astype(np.float16)
    in_maps = []
    for c in range(ncores):
        bs = slice(c * bl, (c + 1) * bl)
        dt_c = np.ascontiguousarray(
            d_all[bs].transpose(2, 1, 0)).reshape(K, t_steps * bl)
        r0 = np.zeros((K, bl), np.float16)
        r0[N:] = inputs[bs, 0].astype(np.float16).T
        in_maps.append({"s_mat": s, "r0": r0, "d": dt_c})
    return in_maps


def gather_out(results, *, t_steps=T, bl=BL, ncores=NCORES):
    out = np.empty((ncores * bl, t_steps, NB), np.float32)
    for c in range(ncores):
        ot = results[c]["out_t"].reshape(NB, t_steps + 1, bl)
        out[c * bl:(c + 1) * bl] = ot[:, 1:, :].transpose(2, 1, 0)
    return out


_NC_CACHE = {}


def kernel(inputs, noise, recurrent_weights, input_weights, output_weights,
           **run_kwargs):
    cfg = run_kwargs.pop("cfg", {})
    key = tuple(sorted(cfg.items()))
    if key not in _NC_CACHE:
        _NC_CACHE[key] = build_nc(**cfg)
    nc = _NC_CACHE[key]
    in_maps = make_in_maps(inputs, noise, recurrent_weights, input_weights,
                           output_weights)
    res = run_bass_kernel_spmd(nc, in_maps, core_ids=list(range(NCORES)),
                               **run_kwargs)
    out = gather_out(res.results)
    if run_kwargs.get("trace"):
        return out, res
    return out
